# revision 56
# baseline (speedup 1.0000x reference)
"""Causal multi-head attention on 8 TRN2 NeuronCores — pipelined bf16.

Sharding: core c -> (batch b = c // 2, head-half hh = c % 2). Each core
computes QKV for its 8 heads over the full sequence of its batch, causal
flash attention, and a partial out-projection; the host sums the two
partials per batch.

Design (measured ~267us; lineage 333.3us fp32r -> 287.6us bf16 -> 267us):
  - Single software-pipelined phase: projection groups for token-chunk c+1
    and out-projections for chunk c-1 run as PE filler inside attention
    chunk c (causality allows it), keeping the PE dense while the ACT
    engine grinds exp() — the attention inner loop alone is ACT-bound.
    Filler is assigned per (chunk, head-pair) with a rotation that never
    writes a KT tile the current head-pair is reading. NOTE: the filler
    balance is delicately tuned — moving outproj(2,3) into j=2's fillers
    or reordering warmup DMAs/projections measured WORSE (the PE and ACT
    queues are in-order; chunk 3 is a serial ACT chain).
  - All matmul operands bf16 (fp32 PSUM accumulation): enables fast weight
    load (fp32r LDWEIGHTS was ~200us of issue time), halves SBUF/DMA, and
    allows exact causal trimming. rel err ~5.6e-3 (gate 2e-2).
  - Both score matmuls share the full 128-row KT[j] stationary (one
    shadow-buffered LDWEIGHTS) and stream zero-padded Q (QT0=[Q_h0;0],
    QT1=[0;Q_h1]): a row_grp=64 partial weight load bypasses the PE's
    shadow bank and serializes against the previous matmul's drain
    (~96ns x 160 = ~15us measured). Matmul time is N (moving dim) only,
    so the padded 128-partition stream costs nothing.
  - V tiles carry per-head [V_h (64) | ones (64)] columns, so the AV
    matmul emits the softmax denominator already replicated across output
    partitions 64:128 (output-partition width is free on the PE) — no
    single-lane copies or gpsimd partition_broadcast in the normalize.
  - HAM clock gate: after ~23us of warmup the HAM grants ~232us of
    full-rate PE, then clamps to 4/8 duty — work scheduled past the
    budget runs at half speed. The final chunk's out-projection is split:
    head pairs 0..1 ship during j=2 (into po), pairs 2..3 are a short
    16-matmul tail into pb, host-summed (see kernel()).
  - Outputs po/pb are bf16 (host sums in fp32): halves the tail-critical
    output DMA; adds ~0.2e-3 rel err.
  - One DMA per SBUF tile (x is tiled per chunk): a split DMA into one
    tile races downstream readers (nondeterministic corruption).

Layouts (per core):
  XS[c][d] [128, 512]  x^T chunk c, rows d*128.. (bf16)
  KT[j]   [128, 2048] K^T head pair j (head 2j rows 0:64, 2j+1 rows 64:128)
  V[t]    [128, 1024] V token-tile t, 8 heads x (64 V cols + 64 ones cols)
  QT0/QT1[g][j] [128, 512] per-chunk Q^T (generation g = chunk%2): QT0 has
    head 2j in rows 0:64 and zeros in 64:128, QT1 zeros then head 2j+1.
    Both score matmuls share the full 128-row KT[j] stationary (one
    shadow-buffered LDWEIGHTS) and stream the zero-padded Q: a row_grp=64
    partial weight load bypasses the PE shadow bank and serializes against
    the previous matmul's drain (~96ns x 160 = ~15us measured).

Shapes (hardcoded): B=4, T=2048, D=1024, H=16, HD=64.
"""
import sys

for _p in ('/opt/trn_rl_repo', '/root/.axon_site/_ro/trn_rl_repo'):
    if _p not in sys.path:
        sys.path.insert(0, _p)

import numpy as np

B, T, D = 4, 2048, 1024
H, HD = 16, 64
HPC = H // 2          # heads per core = 8
DPC = HPC * HD        # out-dims per core = 512
N_CORES = 8

_nc_cache = {}


def _build_nc():
    import concourse.bacc as bacc
    import concourse.mybir as mybir
    from concourse.tile import TileContext

    F32 = mybir.dt.float32
    BF16 = mybir.dt.bfloat16
    AF = mybir.ActivationFunctionType
    ALU = mybir.AluOpType

    QC = 512              # query/token chunk
    NKB = T // 128        # 16 k-blocks
    NQC = T // QC         # 4 chunks
    NDT = D // 128        # 8 input-dim tiles
    # V tile: per head [V_h (64 cols) | ones (64 cols)] so the AV matmul
    # emits the softmax denominator replicated across out partitions
    # 64:128 — no gpsimd partition_broadcast needed downstream.
    VW = HPC * 128        # 1024

    nc = bacc.Bacc('TRN2', target_bir_lowering=False, debug=False)
    xT_d = nc.dram_tensor('xT', [D, T], BF16, kind='ExternalInput')
    wq_d = nc.dram_tensor('wq', [D, DPC], BF16, kind='ExternalInput')
    wk_d = nc.dram_tensor('wk', [D, DPC], BF16, kind='ExternalInput')
    wv_d = nc.dram_tensor('wv', [D, DPC], BF16, kind='ExternalInput')
    wo_d = nc.dram_tensor('wo', [DPC, D], BF16, kind='ExternalInput')
    po_d = nc.dram_tensor('po', [T, D], BF16, kind='ExternalOutput')
    # second partial for the LAST chunk's out-projection (head pairs 2..3),
    # summed into po on the host: lets head pairs 0..1 ship during j=2 and
    # cuts the end-of-kernel tail (which runs under the HAM 4/8 throttle)
    # from 32 matmuls + 2MB DMA to 16 matmuls + overlapped DMA.
    pb_d = nc.dram_tensor('pb', [QC, D], BF16, kind='ExternalOutput')

    with nc.allow_low_precision(reason='bf16 matmuls by design'), \
            TileContext(nc) as tc:
        with (
            tc.tile_pool(name='xs', bufs=1) as xs_pool,
            tc.tile_pool(name='w', bufs=1) as w_pool,
            tc.tile_pool(name='kt', bufs=1) as kt_pool,
            tc.tile_pool(name='vv', bufs=1) as v_pool,
            tc.tile_pool(name='qt', bufs=1) as qt_pool,
            tc.tile_pool(name='pt', bufs=4) as pt_pool,
            tc.tile_pool(name='ao', bufs=2) as ao_pool,
            tc.tile_pool(name='osb', bufs=2) as osb_pool,
            tc.tile_pool(name='small', bufs=2) as sm_pool,
            tc.tile_pool(name='ps_s', bufs=2, space='PSUM') as ps_s,
            tc.tile_pool(name='ps_ot', bufs=2, space='PSUM') as ps_ot,
            tc.tile_pool(name='ps_pj', bufs=2, space='PSUM') as ps_pj,
        ):
            # one tile per (chunk, d-block): each is filled by exactly one
            # DMA so readers wait on a whole-tile transfer (a split DMA
            # into one tile raced its readers under profiling)
            XS = [[xs_pool.tile([128, QC], BF16, tag=f'x{cc}_{d}',
                                name=f'xs{cc}_{d}') for d in range(NDT)]
                  for cc in range(NQC)]
            WK = [w_pool.tile([128, DPC], BF16, tag=f'wk{d}', name=f'wks{d}')
                  for d in range(NDT)]
            WV = [w_pool.tile([128, DPC], BF16, tag=f'wv{d}', name=f'wvs{d}')
                  for d in range(NDT)]
            WQ = [w_pool.tile([128, DPC], BF16, tag=f'wq{d}', name=f'wqs{d}')
                  for d in range(NDT)]
            WO = [w_pool.tile([128, D], BF16, tag=f'wo{d}', name=f'wos{d}')
                  for d in range(4)]
            KT = [kt_pool.tile([128, T], BF16, tag=f'kt{j}', name=f'kt{j}')
                  for j in range(4)]
            V = [v_pool.tile([128, VW], BF16, tag=f'v{t}', name=f'v{t}')
                 for t in range(NKB)]
            # zero-padded per-chunk Q^T, generation g = chunk % 2
            QT0 = [[qt_pool.tile([128, QC], BF16, tag=f'qt0_{g}_{j}',
                                 name=f'qt0_{g}_{j}') for j in range(4)]
                   for g in range(2)]
            QT1 = [[qt_pool.tile([128, QC], BF16, tag=f'qt1_{g}_{j}',
                                 name=f'qt1_{g}_{j}') for j in range(4)]
                   for g in range(2)]

            # pre-warm the ACT exp table + gpsimd paths (no data deps, so
            # these run during the initial DMA wait)
            warm = sm_pool.tile([1, 16], F32, tag='warm', bufs=1)
            nc.vector.memset(warm[:, :], 0.0)
            nc.scalar.activation(warm[:, :], warm[:, :], AF.Exp)
            nc.gpsimd.affine_select(
                out=warm[:, :], in_=warm[:, :], compare_op=ALU.is_ge,
                fill=0.0, base=0, channel_multiplier=-1, pattern=[[1, 16]])
            # zero the dead halves of the padded Q tiles once (gpsimd: the
            # DVE queue is the chunk-0 critical path); generation 0 first --
            # its tiles are read earliest
            for g in range(2):
                for j in range(4):
                    nc.gpsimd.memset(QT0[g][j][64:128, :], 0.0)
                    nc.gpsimd.memset(QT1[g][j][0:64, :], 0.0)
            # seed V tiles with ones; the V projection overwrites each
            # head's first 64 columns, leaving ones in the denominator
            # half. Only chunk-0's k-tiles are needed before the first AV
            # matmul -- the rest are seeded after the prologue emissions so
            # they don't stall chunk-0's KT/Q copies behind 14us of DVE
            # memsets.
            for t in range(4):
                nc.vector.memset(V[t][:, :], 1.0)

            # DMAs ordered so the first projection group's inputs land
            # first. The Sync queue issues descriptors serially (~600ns
            # each), so warmup is DMA-ISSUE-bound with 52 inputs on one
            # queue; the Activation engine is the second hardware DGE and
            # idles through warmup -- WV and XS[1] (16 descriptors, ~10us)
            # issue from it in parallel. No more than that, or the first
            # real exp (~12us in) queues behind the descriptors.
            for d in range(NDT):
                nc.sync.dma_start(WK[d][:, :], wk_d[d*128:(d+1)*128, :])
                nc.sync.dma_start(XS[0][d][:, :],
                                  xT_d[d*128:(d+1)*128, 0:QC])
            for d in range(NDT):
                nc.scalar.dma_start(WV[d][:, :], wv_d[d*128:(d+1)*128, :])
            for d in range(NDT):
                nc.sync.dma_start(WQ[d][:, :], wq_d[d*128:(d+1)*128, :])
            for d in range(NDT):
                nc.scalar.dma_start(XS[1][d][:, :],
                                    xT_d[d*128:(d+1)*128, QC:2*QC])
            for cc in range(2, NQC):
                for d in range(NDT):
                    nc.sync.dma_start(
                        XS[cc][d][:, :],
                        xT_d[d*128:(d+1)*128, cc*QC:(cc+1)*QC])
            for d in range(4):
                nc.sync.dma_start(WO[d][:, :], wo_d[d*128:(d+1)*128, :])

            # ---------------- emission helpers ----------------
            def emit_proj_kt(c, j):
                pp = ps_pj.tile([128, QC], F32, tag='pj', name=f'pk{c}{j}')
                for d in range(NDT):
                    nc.tensor.matmul(
                        pp[:, :], lhsT=WK[d][:, j*128:(j+1)*128],
                        rhs=XS[c][d][:, :],
                        start=(d == 0), stop=(d == NDT - 1))
                nc.vector.tensor_copy(KT[j][:, c*QC:(c+1)*QC], pp[:, :])

            def emit_proj_v(c, tt):
                t = c * 4 + tt
                pv = ps_pj.tile([128, DPC], F32, tag='pj', name=f'pv{t}')
                for d in range(NDT):
                    nc.tensor.matmul(
                        pv[:, :], lhsT=XS[c][d][:, tt*128:(tt+1)*128],
                        rhs=WV[d][:, :],
                        start=(d == 0), stop=(d == NDT - 1))
                vt3 = V[t].rearrange('p (h c) -> p h c', c=128)
                nc.vector.tensor_copy(
                    vt3[:, :, 0:HD], pv.rearrange('p (h c) -> p h c', c=HD))

            def emit_proj_q(c, j):
                pq = ps_pj.tile([128, QC], F32, tag='pj', name=f'pq{c}{j}')
                for d in range(NDT):
                    nc.tensor.matmul(
                        pq[:, :], lhsT=WQ[d][:, j*128:(j+1)*128],
                        rhs=XS[c][d][:, :],
                        start=(d == 0), stop=(d == NDT - 1))
                g = c % 2
                nc.vector.tensor_copy(QT0[g][j][0:64, :], pq[0:64, :])
                nc.vector.tensor_copy(QT1[g][j][64:128, :], pq[64:128, :])

            AOs = {}

            def emit_outproj(c, qt):
                q0 = c * QC
                ao = AOs[c]
                os = osb_pool.tile([128, D], BF16, tag='os', name=f'os{c}{qt}')
                for half in range(2):
                    pj = ps_pj.tile([128, 512], F32, tag='pj',
                                    name=f'po{c}{qt}{half}')
                    for d in range(4):
                        nc.tensor.matmul(
                            pj[:, :],
                            lhsT=ao[d][:, qt*128:(qt+1)*128],
                            rhs=WO[d][:, half*512:(half+1)*512],
                            start=(d == 0), stop=(d == 3))
                    nc.vector.tensor_copy(
                        os[:, half*512:(half+1)*512], pj[:, :])
                nc.sync.dma_start(
                    po_d[q0+qt*128:q0+(qt+1)*128, :], os[:, :])

            def emit_final_outproj_half(ao, j, qts=(0, 1, 2, 3)):
                # partial out-projection for the LAST chunk over head pairs
                # (j-1, j): at j==2 ship pairs 0..1 into po rows 1536:2048;
                # at j==3 ship pairs 2..3 into pb (host adds the two). On
                # the j==3 tail the PSUM->SBUF staging casts rotate across
                # scalar/gpsimd/vector (all otherwise idle there) so the
                # copy chain isn't serialized on the DVE behind the last
                # normalize.
                dst = po_d[3*QC:4*QC, :] if j == 2 else pb_d
                dpair = (0, 1) if j == 2 else (2, 3)
                for qt in qts:
                    os = osb_pool.tile([128, D], BF16, tag='os',
                                       name=f'osf{j}{qt}')
                    for half in range(2):
                        pj = ps_pj.tile([128, 512], F32, tag='pj',
                                        name=f'pof{j}{qt}{half}')
                        for d in dpair:
                            nc.tensor.matmul(
                                pj[:, :],
                                lhsT=ao[d][:, qt*128:(qt+1)*128],
                                rhs=WO[d][:, half*512:(half+1)*512],
                                start=(d == dpair[0]), stop=(d == dpair[1]))
                        oslice = os[:, half*512:(half+1)*512]
                        if j == 3 and (2 * qt + half) % 2 == 0:
                            nc.scalar.activation(oslice, pj[:, :], AF.Copy)
                        else:
                            nc.vector.tensor_copy(oslice, pj[:, :])
                        nc.sync.dma_start(
                            dst[qt*128:(qt+1)*128, half*512:(half+1)*512],
                            oslice)

            # ---------------- prologue: minimal chunk-0 set ----------------
            emit_proj_kt(0, 0)
            for tt in range(4):
                emit_proj_v(0, tt)
            emit_proj_q(0, 0)
            # remaining V ones-seeds: DVE is idle while chunk-0 QK runs, and
            # DVE program order puts these before the V projections of
            # chunks 1..3 (emitted later as filler) that overwrite the data
            # halves
            for t in range(4, NKB):
                nc.vector.memset(V[t][:, :], 1.0)

            # ------------- per-(chunk, head-pair) filler map -------------
            # Rotation rule: KT(c', jx) is never pumped during attn(c, jx)
            # (same-tile write/read), and lands one head-pair before its
            # first reader.
            def filler_map(c):
                f = {0: [], 1: [], 2: [], 3: []}
                if c == 0:
                    for j in range(4):
                        if j < 3:
                            f[j] += [lambda j=j: emit_proj_kt(0, j + 1),
                                     lambda j=j: emit_proj_q(0, j + 1)]
                        f[j] += [lambda j=j: emit_proj_kt(1, (j + 1) % 4),
                                 lambda j=j: emit_proj_v(1, j)]
                    f[3] += [lambda: emit_proj_q(1, 0),
                             lambda: emit_proj_q(1, 1),
                             lambda: emit_proj_q(1, 2),
                             lambda: emit_proj_q(1, 3)]
                elif c == 1:
                    for j in range(4):
                        f[j] += [lambda j=j: emit_proj_kt(2, (j + 1) % 4),
                                 lambda j=j: emit_proj_q(2, (j + 1) % 4),
                                 lambda j=j: emit_proj_v(2, j),
                                 lambda j=j: emit_outproj(0, j)]
                elif c == 2:
                    for j in range(4):
                        f[j] += [lambda j=j: emit_proj_v(3, j),
                                 lambda j=j: emit_outproj(1, j)]
                    f[3] += [lambda: emit_proj_kt(3, 0),
                             lambda: emit_proj_q(3, 0)]
                else:
                    for j in range(3):
                        f[j] += [lambda j=j: emit_proj_kt(3, j + 1),
                                 lambda j=j: emit_proj_q(3, j + 1),
                                 lambda j=j: emit_outproj(2, j)]
                    f[3] += [lambda: emit_outproj(2, 3)]
                return f

            # ---------------- pipelined chunk loop ----------------
            for c in range(NQC):
                q0 = c * QC
                nkb = (q0 + QC) // 128
                g = c % 2
                fmap = filler_map(c)
                ao = [ao_pool.tile([128, QC], BF16, tag=f'ao{j}',
                                   name=f'ao{j}c{c}') for j in range(4)]
                AOs[c] = ao
                for j in range(4):            # head pair (2j, 2j+1)
                    h0, h1 = 2*j, 2*j + 1
                    filler = fmap[j]
                    slots = nkb // 2
                    emitted = 0
                    slot = 0
                    ot0 = ps_ot.tile([128, QC], F32, tag='ot', name='ot0')
                    ot1 = ps_ot.tile([128, QC], F32, tag='ot', name='ot1')
                    pend = None
                    for kbp in range(slots):
                        ka, kB = 2*kbp, 2*kbp + 1
                        lo_a = max(0, ka*128 - q0)
                        lo_b = max(0, kB*128 - q0)
                        s0 = ps_s.tile([128, 2*QC], F32, tag='s', name='s0')
                        s1 = ps_s.tile([128, 2*QC], F32, tag='s', name='s1')
                        pt0 = pt_pool.tile([128, 2*QC], BF16, tag='pt',
                                           name='pt0')
                        pt1 = pt_pool.tile([128, 2*QC], BF16, tag='pt',
                                           name='pt1')
                        ksa = KT[j][:, ka*128:(ka+1)*128]
                        ksb = KT[j][:, kB*128:(kB+1)*128]
                        # full 128-row stationary shared by both heads; the
                        # dead half of the zero-padded Q contributes exactly 0
                        nc.tensor.matmul(
                            s0[:, lo_a:QC], lhsT=ksa[:, :],
                            rhs=QT0[g][j][:, lo_a:QC],
                            start=True, stop=True)
                        nc.tensor.matmul(
                            s1[:, lo_a:QC], lhsT=ksa[:, :],
                            rhs=QT1[g][j][:, lo_a:QC],
                            start=True, stop=True)
                        nc.tensor.matmul(
                            s0[:, QC+lo_b:2*QC], lhsT=ksb[:, :],
                            rhs=QT0[g][j][:, lo_b:QC],
                            start=True, stop=True)
                        nc.tensor.matmul(
                            s1[:, QC+lo_b:2*QC], lhsT=ksb[:, :],
                            rhs=QT1[g][j][:, lo_b:QC],
                            start=True, stop=True)
                        if pend is not None:
                            for (pk, pl, pc0), ppt in pend:
                                nc.tensor.matmul(
                                    ot0[:, pl:QC],
                                    lhsT=V[pk][:, 128*h0:128*(h0+1)],
                                    rhs=ppt[0][:, pc0+pl:pc0+QC],
                                    start=(pk == 0), stop=False)
                                nc.tensor.matmul(
                                    ot1[:, pl:QC],
                                    lhsT=V[pk][:, 128*h1:128*(h1+1)],
                                    rhs=ppt[1][:, pc0+pl:pc0+QC],
                                    start=(pk == 0), stop=False)
                        nc.scalar.activation(
                            pt0[:, lo_a:2*QC], s0[:, lo_a:2*QC], AF.Exp)
                        nc.scalar.activation(
                            pt1[:, lo_a:2*QC], s1[:, lo_a:2*QC], AF.Exp)
                        for kx, lox, c0 in ((ka, lo_a, 0), (kB, lo_b, QC)):
                            if kx*128 >= q0:   # causal mask on diag block
                                for ptx in (pt0, pt1):
                                    nc.gpsimd.affine_select(
                                        out=ptx[:, c0+lox:c0+lox+128],
                                        in_=ptx[:, c0+lox:c0+lox+128],
                                        compare_op=ALU.is_ge, fill=0.0,
                                        base=0,
                                        channel_multiplier=-1,
                                        pattern=[[1, 128]])
                        pend = [((ka, lo_a, 0), (pt0, pt1)),
                                ((kB, lo_b, QC), (pt0, pt1))]
                        slot += 1
                        want = (slot * len(filler)) // slots
                        while emitted < want:
                            filler[emitted]()
                            emitted += 1
                    for (pk, pl, pc0), ppt in pend:
                        nc.tensor.matmul(
                            ot0[:, pl:QC],
                            lhsT=V[pk][:, 128*h0:128*(h0+1)],
                            rhs=ppt[0][:, pc0+pl:pc0+QC],
                            start=(pk == 0), stop=(pk == nkb - 1))
                        nc.tensor.matmul(
                            ot1[:, pl:QC],
                            lhsT=V[pk][:, 128*h1:128*(h1+1)],
                            rhs=ppt[1][:, pc0+pl:pc0+QC],
                            start=(pk == 0), stop=(pk == nkb - 1))
                    # normalize both heads of the pair; ot rows 64:128
                    # hold the denominator already replicated 64x. For the
                    # final head pair the chain runs in column halves so the
                    # tail's partial out-projection matmuls start after half
                    # 0 instead of waiting out the full serial DVE chain.
                    final3 = (c == NQC - 1 and j == 3)
                    dsb0 = sm_pool.tile([HD, QC], F32, tag='dsb0', bufs=2)
                    dsb1 = sm_pool.tile([HD, QC], F32, tag='dsb1', bufs=2)
                    rsb0 = sm_pool.tile([HD, QC], F32, tag='rsb0', bufs=2)
                    rsb1 = sm_pool.tile([HD, QC], F32, tag='rsb1', bufs=2)
                    spans = (((0, QC // 2), (QC // 2, QC)) if final3
                             else ((0, QC),))
                    for si, (a, b) in enumerate(spans):
                        nc.vector.tensor_copy(dsb0[:, a:b], ot0[HD:128, a:b])
                        nc.vector.tensor_copy(dsb1[:, a:b], ot1[HD:128, a:b])
                        nc.vector.reciprocal_approx_fast(
                            out=rsb0[:, a:b], in_=dsb0[:, a:b])
                        nc.vector.reciprocal_approx_fast(
                            out=rsb1[:, a:b], in_=dsb1[:, a:b])
                        nc.vector.tensor_tensor(
                            out=ao[j][0:HD, a:b], in0=ot0[0:HD, a:b],
                            in1=rsb0[:, a:b], op=ALU.mult)
                        nc.vector.tensor_tensor(
                            out=ao[j][HD:128, a:b], in0=ot1[0:HD, a:b],
                            in1=rsb1[:, a:b], op=ALU.mult)
                        if final3:
                            emit_final_outproj_half(
                                ao, 3, qts=(0, 1) if si == 0 else (2, 3))
                    while emitted < len(filler):
                        filler[emitted]()
                        emitted += 1
                    if c == NQC - 1 and j == 2:
                        emit_final_outproj_half(ao, 2)
            # final chunk's out-projection is emitted inside the j==2/j==3
            # iterations via emit_final_outproj_half

    nc.compile()
    return nc


def _get_nc():
    if 'nc' not in _nc_cache:
        _nc_cache['nc'] = _build_nc()
    return _nc_cache['nc']


def kernel(x, w_qkv, w_out, _profile=False):
    from concourse.bass_utils import run_bass_kernel_spmd
    import ml_dtypes

    bf16 = ml_dtypes.bfloat16
    x = np.asarray(x, dtype=np.float32)
    w_qkv = np.asarray(w_qkv, dtype=np.float32)
    w_out = np.asarray(w_out, dtype=np.float32)

    nc = _get_nc()

    scale = np.float32(1.0 / np.sqrt(HD))
    in_maps = []
    for c in range(N_CORES):
        b, hh = c // 2, c % 2
        s, e = hh * DPC, (hh + 1) * DPC
        in_maps.append({
            'xT': np.ascontiguousarray(x[b].T).astype(bf16),
            'wq': np.ascontiguousarray(w_qkv[:, s:e] * scale).astype(bf16),
            'wk': np.ascontiguousarray(w_qkv[:, D+s:D+e]).astype(bf16),
            'wv': np.ascontiguousarray(w_qkv[:, 2*D+s:2*D+e]).astype(bf16),
            'wo': np.ascontiguousarray(w_out[s:e, :]).astype(bf16),
        })

    res = run_bass_kernel_spmd(nc, in_maps, core_ids=list(range(N_CORES)),
                               trace=_profile)
    out = np.empty((B, T, D), np.float32)
    for b in range(B):
        r0, r1 = res.results[2*b], res.results[2*b+1]
        out[b] = r0['po'].astype(np.float32) + r1['po'].astype(np.float32)
        out[b][T - 512:] += r0['pb'].astype(np.float32) + r1['pb'].astype(np.float32)
    if _profile:
        return out, res
    return out



# revision 57
# speedup vs baseline: 1.0193x; 1.0193x over previous
"""Causal multi-head attention on 8 TRN2 NeuronCores — pipelined bf16.

Sharding: core c -> (batch b = c // 2, head-half hh = c % 2). Each core
computes QKV for its 8 heads over the full sequence of its batch, causal
flash attention, and a partial out-projection; the host sums the two
partials per batch.

Design (measured ~267us; lineage 333.3us fp32r -> 287.6us bf16 -> 267us):
  - Single software-pipelined phase: projection groups for token-chunk c+1
    and out-projections for chunk c-1 run as PE filler inside attention
    chunk c (causality allows it), keeping the PE dense while the ACT
    engine grinds exp() — the attention inner loop alone is ACT-bound.
    Filler is assigned per (chunk, head-pair) with a rotation that never
    writes a KT tile the current head-pair is reading. NOTE: the filler
    balance is delicately tuned — moving outproj(2,3) into j=2's fillers
    or reordering warmup DMAs/projections measured WORSE (the PE and ACT
    queues are in-order; chunk 3 is a serial ACT chain).
  - All matmul operands bf16 (fp32 PSUM accumulation): enables fast weight
    load (fp32r LDWEIGHTS was ~200us of issue time), halves SBUF/DMA, and
    allows exact causal trimming. rel err ~5.6e-3 (gate 2e-2).
  - Both score matmuls share the full 128-row KT[j] stationary (one
    shadow-buffered LDWEIGHTS) and stream zero-padded Q (QT0=[Q_h0;0],
    QT1=[0;Q_h1]): a row_grp=64 partial weight load bypasses the PE's
    shadow bank and serializes against the previous matmul's drain
    (~96ns x 160 = ~15us measured). Matmul time is N (moving dim) only,
    so the padded 128-partition stream costs nothing.
  - V tiles carry per-head [V_h (64) | ones (64)] columns, so the AV
    matmul emits the softmax denominator already replicated across output
    partitions 64:128 (output-partition width is free on the PE) — no
    single-lane copies or gpsimd partition_broadcast in the normalize.
  - HAM clock gate: after ~23us of warmup the HAM grants ~232us of
    full-rate PE, then clamps to 4/8 duty — work scheduled past the
    budget runs at half speed. The final chunk's out-projection is split:
    head pairs 0..1 ship during j=2 (into po), pairs 2..3 are a short
    16-matmul tail into pb, host-summed (see kernel()).
  - Outputs po/pb are bf16 (host sums in fp32): halves the tail-critical
    output DMA; adds ~0.2e-3 rel err.
  - One DMA per SBUF tile (x is tiled per chunk): a split DMA into one
    tile races downstream readers (nondeterministic corruption).

Layouts (per core):
  XS[c][d] [128, 512]  x^T chunk c, rows d*128.. (bf16)
  KT[j]   [128, 2048] K^T head pair j (head 2j rows 0:64, 2j+1 rows 64:128)
  V[t]    [128, 1024] V token-tile t, 8 heads x (64 V cols + 64 ones cols)
  QT0/QT1[g][j] [128, 512] per-chunk Q^T (generation g = chunk%2): QT0 has
    head 2j in rows 0:64 and zeros in 64:128, QT1 zeros then head 2j+1.
    Both score matmuls share the full 128-row KT[j] stationary (one
    shadow-buffered LDWEIGHTS) and stream the zero-padded Q: a row_grp=64
    partial weight load bypasses the PE shadow bank and serializes against
    the previous matmul's drain (~96ns x 160 = ~15us measured).

Shapes (hardcoded): B=4, T=2048, D=1024, H=16, HD=64.
"""
import sys

for _p in ('/opt/trn_rl_repo', '/root/.axon_site/_ro/trn_rl_repo'):
    if _p not in sys.path:
        sys.path.insert(0, _p)

import numpy as np

B, T, D = 4, 2048, 1024
H, HD = 16, 64
HPC = H // 2          # heads per core = 8
DPC = HPC * HD        # out-dims per core = 512
N_CORES = 8

_nc_cache = {}


def _build_nc():
    import concourse.bacc as bacc
    import concourse.mybir as mybir
    from concourse.tile import TileContext

    F32 = mybir.dt.float32
    BF16 = mybir.dt.bfloat16
    AF = mybir.ActivationFunctionType
    ALU = mybir.AluOpType

    QC = 512              # query/token chunk
    NKB = T // 128        # 16 k-blocks
    NQC = T // QC         # 4 chunks
    NDT = D // 128        # 8 input-dim tiles
    # V tile: per head [V_h (64 cols) | ones (64 cols)] so the AV matmul
    # emits the softmax denominator replicated across out partitions
    # 64:128 — no gpsimd partition_broadcast needed downstream.
    VW = HPC * 128        # 1024

    nc = bacc.Bacc('TRN2', target_bir_lowering=False, debug=False)
    xT_d = nc.dram_tensor('xT', [D, T], BF16, kind='ExternalInput')
    wq_d = nc.dram_tensor('wq', [D, DPC], BF16, kind='ExternalInput')
    wk_d = nc.dram_tensor('wk', [D, DPC], BF16, kind='ExternalInput')
    wv_d = nc.dram_tensor('wv', [D, DPC], BF16, kind='ExternalInput')
    wo_d = nc.dram_tensor('wo', [DPC, D], BF16, kind='ExternalInput')
    po_d = nc.dram_tensor('po', [T, D], BF16, kind='ExternalOutput')
    # second partial for the LAST chunk's out-projection (head pairs 2..3),
    # summed into po on the host: lets head pairs 0..1 ship during j=2 and
    # cuts the end-of-kernel tail (which runs under the HAM 4/8 throttle)
    # from 32 matmuls + 2MB DMA to 16 matmuls + overlapped DMA.
    pb_d = nc.dram_tensor('pb', [QC, D], BF16, kind='ExternalOutput')

    with nc.allow_low_precision(reason='bf16 matmuls by design'), \
            TileContext(nc) as tc:
        with (
            tc.tile_pool(name='xs', bufs=1) as xs_pool,
            tc.tile_pool(name='w', bufs=1) as w_pool,
            tc.tile_pool(name='kt', bufs=1) as kt_pool,
            tc.tile_pool(name='vv', bufs=1) as v_pool,
            tc.tile_pool(name='qt', bufs=1) as qt_pool,
            tc.tile_pool(name='pt', bufs=4) as pt_pool,
            tc.tile_pool(name='ao', bufs=2) as ao_pool,
            tc.tile_pool(name='osb', bufs=2) as osb_pool,
            tc.tile_pool(name='small', bufs=2) as sm_pool,
            tc.tile_pool(name='ps_s', bufs=2, space='PSUM') as ps_s,
            tc.tile_pool(name='ps_ot', bufs=2, space='PSUM') as ps_ot,
            tc.tile_pool(name='ps_pj', bufs=2, space='PSUM') as ps_pj,
        ):
            # one tile per (chunk, d-block): each is filled by exactly one
            # DMA so readers wait on a whole-tile transfer (a split DMA
            # into one tile raced its readers under profiling)
            XS = [[xs_pool.tile([128, QC], BF16, tag=f'x{cc}_{d}',
                                name=f'xs{cc}_{d}') for d in range(NDT)]
                  for cc in range(NQC)]
            WK = [w_pool.tile([128, DPC], BF16, tag=f'wk{d}', name=f'wks{d}')
                  for d in range(NDT)]
            WV = [w_pool.tile([128, DPC], BF16, tag=f'wv{d}', name=f'wvs{d}')
                  for d in range(NDT)]
            WQ = [w_pool.tile([128, DPC], BF16, tag=f'wq{d}', name=f'wqs{d}')
                  for d in range(NDT)]
            WO = [w_pool.tile([128, D], BF16, tag=f'wo{d}', name=f'wos{d}')
                  for d in range(4)]
            KT = [kt_pool.tile([128, T], BF16, tag=f'kt{j}', name=f'kt{j}')
                  for j in range(4)]
            V = [v_pool.tile([128, VW], BF16, tag=f'v{t}', name=f'v{t}')
                 for t in range(NKB)]
            # zero-padded per-chunk Q^T, generation g = chunk % 2
            QT0 = [[qt_pool.tile([128, QC], BF16, tag=f'qt0_{g}_{j}',
                                 name=f'qt0_{g}_{j}') for j in range(4)]
                   for g in range(2)]
            QT1 = [[qt_pool.tile([128, QC], BF16, tag=f'qt1_{g}_{j}',
                                 name=f'qt1_{g}_{j}') for j in range(4)]
                   for g in range(2)]

            # pre-warm the ACT exp table + gpsimd paths (no data deps, so
            # these run during the initial DMA wait)
            warm = sm_pool.tile([1, 16], F32, tag='warm', bufs=1)
            nc.vector.memset(warm[:, :], 0.0)
            nc.scalar.activation(warm[:, :], warm[:, :], AF.Exp)
            nc.gpsimd.affine_select(
                out=warm[:, :], in_=warm[:, :], compare_op=ALU.is_ge,
                fill=0.0, base=0, channel_multiplier=-1, pattern=[[1, 16]])
            # zero the dead halves of the padded Q tiles once (gpsimd: the
            # DVE queue is the chunk-0 critical path); generation 0 first --
            # its tiles are read earliest
            for g in range(2):
                for j in range(4):
                    nc.gpsimd.memset(QT0[g][j][64:128, :], 0.0)
                    nc.gpsimd.memset(QT1[g][j][0:64, :], 0.0)
            # seed V tiles with ones; the V projection overwrites each
            # head's first 64 columns, leaving ones in the denominator
            # half. Only chunk-0's k-tiles are needed before the first AV
            # matmul -- the rest are seeded after the prologue emissions so
            # they don't stall chunk-0's KT/Q copies behind 14us of DVE
            # memsets.
            for t in range(4):
                nc.vector.memset(V[t][:, :], 1.0)

            # DMAs ordered so the first projection group's inputs land first
            for d in range(NDT):
                nc.sync.dma_start(WK[d][:, :], wk_d[d*128:(d+1)*128, :])
                nc.sync.dma_start(XS[0][d][:, :],
                                  xT_d[d*128:(d+1)*128, 0:QC])
            for d in range(NDT):
                nc.sync.dma_start(WV[d][:, :], wv_d[d*128:(d+1)*128, :])
            for d in range(NDT):
                nc.sync.dma_start(WQ[d][:, :], wq_d[d*128:(d+1)*128, :])
            for cc in range(1, NQC):
                for d in range(NDT):
                    nc.sync.dma_start(
                        XS[cc][d][:, :],
                        xT_d[d*128:(d+1)*128, cc*QC:(cc+1)*QC])
            for d in range(4):
                nc.sync.dma_start(WO[d][:, :], wo_d[d*128:(d+1)*128, :])

            # ---------------- emission helpers ----------------
            def emit_proj_kt(c, j):
                pp = ps_pj.tile([128, QC], F32, tag='pj', name=f'pk{c}{j}')
                for d in range(NDT):
                    nc.tensor.matmul(
                        pp[:, :], lhsT=WK[d][:, j*128:(j+1)*128],
                        rhs=XS[c][d][:, :],
                        start=(d == 0), stop=(d == NDT - 1))
                nc.vector.tensor_copy(KT[j][:, c*QC:(c+1)*QC], pp[:, :])

            def emit_proj_v(c, tt):
                t = c * 4 + tt
                pv = ps_pj.tile([128, DPC], F32, tag='pj', name=f'pv{t}')
                for d in range(NDT):
                    nc.tensor.matmul(
                        pv[:, :], lhsT=XS[c][d][:, tt*128:(tt+1)*128],
                        rhs=WV[d][:, :],
                        start=(d == 0), stop=(d == NDT - 1))
                vt3 = V[t].rearrange('p (h c) -> p h c', c=128)
                nc.vector.tensor_copy(
                    vt3[:, :, 0:HD], pv.rearrange('p (h c) -> p h c', c=HD))

            def emit_proj_q(c, j):
                pq = ps_pj.tile([128, QC], F32, tag='pj', name=f'pq{c}{j}')
                for d in range(NDT):
                    nc.tensor.matmul(
                        pq[:, :], lhsT=WQ[d][:, j*128:(j+1)*128],
                        rhs=XS[c][d][:, :],
                        start=(d == 0), stop=(d == NDT - 1))
                g = c % 2
                nc.vector.tensor_copy(QT0[g][j][0:64, :], pq[0:64, :])
                nc.vector.tensor_copy(QT1[g][j][64:128, :], pq[64:128, :])

            AOs = {}

            def emit_outproj(c, qt):
                q0 = c * QC
                ao = AOs[c]
                os = osb_pool.tile([128, D], BF16, tag='os', name=f'os{c}{qt}')
                for half in range(2):
                    pj = ps_pj.tile([128, 512], F32, tag='pj',
                                    name=f'po{c}{qt}{half}')
                    for d in range(4):
                        nc.tensor.matmul(
                            pj[:, :],
                            lhsT=ao[d][:, qt*128:(qt+1)*128],
                            rhs=WO[d][:, half*512:(half+1)*512],
                            start=(d == 0), stop=(d == 3))
                    nc.vector.tensor_copy(
                        os[:, half*512:(half+1)*512], pj[:, :])
                nc.sync.dma_start(
                    po_d[q0+qt*128:q0+(qt+1)*128, :], os[:, :])

            def emit_final_outproj_half(ao, j, qts=(0, 1, 2, 3)):
                # partial out-projection for the LAST chunk over head pairs
                # (j-1, j): at j==2 ship pairs 0..1 into po rows 1536:2048;
                # at j==3 ship pairs 2..3 into pb (host adds the two). On
                # the j==3 tail the PSUM->SBUF staging casts rotate across
                # scalar/gpsimd/vector (all otherwise idle there) so the
                # copy chain isn't serialized on the DVE behind the last
                # normalize.
                dst = po_d[3*QC:4*QC, :] if j == 2 else pb_d
                dpair = (0, 1) if j == 2 else (2, 3)
                for qt in qts:
                    os = osb_pool.tile([128, D], BF16, tag='os',
                                       name=f'osf{j}{qt}')
                    for half in range(2):
                        pj = ps_pj.tile([128, 512], F32, tag='pj',
                                        name=f'pof{j}{qt}{half}')
                        for d in dpair:
                            nc.tensor.matmul(
                                pj[:, :],
                                lhsT=ao[d][:, qt*128:(qt+1)*128],
                                rhs=WO[d][:, half*512:(half+1)*512],
                                start=(d == dpair[0]), stop=(d == dpair[1]))
                        oslice = os[:, half*512:(half+1)*512]
                        if j == 3 and (2 * qt + half) % 2 == 0:
                            nc.scalar.activation(oslice, pj[:, :], AF.Copy)
                        else:
                            nc.vector.tensor_copy(oslice, pj[:, :])
                        nc.sync.dma_start(
                            dst[qt*128:(qt+1)*128, half*512:(half+1)*512],
                            oslice)

            # ---------------- prologue: minimal chunk-0 set ----------------
            emit_proj_kt(0, 0)
            for tt in range(4):
                emit_proj_v(0, tt)
            emit_proj_q(0, 0)
            # remaining V ones-seeds: DVE is idle while chunk-0 QK runs, and
            # DVE program order puts these before the V projections of
            # chunks 1..3 (emitted later as filler) that overwrite the data
            # halves
            for t in range(4, NKB):
                nc.vector.memset(V[t][:, :], 1.0)

            # ------------- per-(chunk, head-pair) filler map -------------
            # Rotation rule: KT(c', jx) is never pumped during attn(c, jx)
            # (same-tile write/read), and lands one head-pair before its
            # first reader.
            def filler_map(c):
                f = {0: [], 1: [], 2: [], 3: []}
                if c == 0:
                    for j in range(4):
                        if j < 3:
                            f[j] += [lambda j=j: emit_proj_kt(0, j + 1),
                                     lambda j=j: emit_proj_q(0, j + 1)]
                        f[j] += [lambda j=j: emit_proj_kt(1, (j + 1) % 4),
                                 lambda j=j: emit_proj_v(1, j)]
                    f[3] += [lambda: emit_proj_q(1, 0),
                             lambda: emit_proj_q(1, 1),
                             lambda: emit_proj_q(1, 2),
                             lambda: emit_proj_q(1, 3)]
                elif c == 1:
                    for j in range(4):
                        f[j] += [lambda j=j: emit_proj_kt(2, (j + 1) % 4),
                                 lambda j=j: emit_proj_q(2, (j + 1) % 4),
                                 lambda j=j: emit_proj_v(2, j),
                                 lambda j=j: emit_outproj(0, j)]
                elif c == 2:
                    for j in range(4):
                        f[j] += [lambda j=j: emit_proj_v(3, j),
                                 lambda j=j: emit_outproj(1, j)]
                    f[3] += [lambda: emit_proj_kt(3, 0),
                             lambda: emit_proj_q(3, 0)]
                else:
                    for j in range(3):
                        f[j] += [lambda j=j: emit_proj_kt(3, j + 1),
                                 lambda j=j: emit_proj_q(3, j + 1),
                                 lambda j=j: emit_outproj(2, j)]
                    f[3] += [lambda: emit_outproj(2, 3)]
                return f

            # ---------------- pipelined chunk loop ----------------
            for c in range(NQC):
                q0 = c * QC
                nkb = (q0 + QC) // 128
                g = c % 2
                fmap = filler_map(c)
                ao = [ao_pool.tile([128, QC], BF16, tag=f'ao{j}',
                                   name=f'ao{j}c{c}') for j in range(4)]
                AOs[c] = ao
                for j in range(4):            # head pair (2j, 2j+1)
                    h0, h1 = 2*j, 2*j + 1
                    filler = fmap[j]
                    slots = nkb // 2
                    emitted = 0
                    slot = 0
                    ot0 = ps_ot.tile([128, QC], F32, tag='ot', name='ot0')
                    ot1 = ps_ot.tile([128, QC], F32, tag='ot', name='ot1')
                    pend = None
                    for kbp in range(slots):
                        ka, kB = 2*kbp, 2*kbp + 1
                        lo_a = max(0, ka*128 - q0)
                        lo_b = max(0, kB*128 - q0)
                        s0 = ps_s.tile([128, 2*QC], F32, tag='s', name='s0')
                        s1 = ps_s.tile([128, 2*QC], F32, tag='s', name='s1')
                        pt0 = pt_pool.tile([128, 2*QC], BF16, tag='pt',
                                           name='pt0')
                        pt1 = pt_pool.tile([128, 2*QC], BF16, tag='pt',
                                           name='pt1')
                        ksa = KT[j][:, ka*128:(ka+1)*128]
                        ksb = KT[j][:, kB*128:(kB+1)*128]
                        # full 128-row stationary shared by both heads; the
                        # dead half of the zero-padded Q contributes exactly 0
                        nc.tensor.matmul(
                            s0[:, lo_a:QC], lhsT=ksa[:, :],
                            rhs=QT0[g][j][:, lo_a:QC],
                            start=True, stop=True)
                        nc.tensor.matmul(
                            s1[:, lo_a:QC], lhsT=ksa[:, :],
                            rhs=QT1[g][j][:, lo_a:QC],
                            start=True, stop=True)
                        nc.tensor.matmul(
                            s0[:, QC+lo_b:2*QC], lhsT=ksb[:, :],
                            rhs=QT0[g][j][:, lo_b:QC],
                            start=True, stop=True)
                        nc.tensor.matmul(
                            s1[:, QC+lo_b:2*QC], lhsT=ksb[:, :],
                            rhs=QT1[g][j][:, lo_b:QC],
                            start=True, stop=True)
                        if pend is not None:
                            for (pk, pl, pc0), ppt in pend:
                                nc.tensor.matmul(
                                    ot0[:, pl:QC],
                                    lhsT=V[pk][:, 128*h0:128*(h0+1)],
                                    rhs=ppt[0][:, pc0+pl:pc0+QC],
                                    start=(pk == 0), stop=False)
                                nc.tensor.matmul(
                                    ot1[:, pl:QC],
                                    lhsT=V[pk][:, 128*h1:128*(h1+1)],
                                    rhs=ppt[1][:, pc0+pl:pc0+QC],
                                    start=(pk == 0), stop=False)
                        nc.scalar.activation(
                            pt0[:, lo_a:2*QC], s0[:, lo_a:2*QC], AF.Exp)
                        nc.scalar.activation(
                            pt1[:, lo_a:2*QC], s1[:, lo_a:2*QC], AF.Exp)
                        for kx, lox, c0 in ((ka, lo_a, 0), (kB, lo_b, QC)):
                            if kx*128 >= q0:   # causal mask on diag block
                                for ptx in (pt0, pt1):
                                    nc.gpsimd.affine_select(
                                        out=ptx[:, c0+lox:c0+lox+128],
                                        in_=ptx[:, c0+lox:c0+lox+128],
                                        compare_op=ALU.is_ge, fill=0.0,
                                        base=0,
                                        channel_multiplier=-1,
                                        pattern=[[1, 128]])
                        pend = [((ka, lo_a, 0), (pt0, pt1)),
                                ((kB, lo_b, QC), (pt0, pt1))]
                        slot += 1
                        want = (slot * len(filler)) // slots
                        while emitted < want:
                            filler[emitted]()
                            emitted += 1
                    for (pk, pl, pc0), ppt in pend:
                        nc.tensor.matmul(
                            ot0[:, pl:QC],
                            lhsT=V[pk][:, 128*h0:128*(h0+1)],
                            rhs=ppt[0][:, pc0+pl:pc0+QC],
                            start=(pk == 0), stop=(pk == nkb - 1))
                        nc.tensor.matmul(
                            ot1[:, pl:QC],
                            lhsT=V[pk][:, 128*h1:128*(h1+1)],
                            rhs=ppt[1][:, pc0+pl:pc0+QC],
                            start=(pk == 0), stop=(pk == nkb - 1))
                    # normalize both heads of the pair; ot rows 64:128
                    # hold the denominator already replicated 64x. For the
                    # final head pair the chain runs in column halves so the
                    # tail's partial out-projection matmuls start after half
                    # 0 instead of waiting out the full serial DVE chain.
                    final3 = (c == NQC - 1 and j == 3)
                    dsb0 = sm_pool.tile([HD, QC], F32, tag='dsb0', bufs=2)
                    dsb1 = sm_pool.tile([HD, QC], F32, tag='dsb1', bufs=2)
                    rsb0 = sm_pool.tile([HD, QC], F32, tag='rsb0', bufs=2)
                    rsb1 = sm_pool.tile([HD, QC], F32, tag='rsb1', bufs=2)
                    spans = (((0, QC // 2), (QC // 2, QC)) if final3
                             else ((0, QC),))
                    for si, (a, b) in enumerate(spans):
                        nc.vector.tensor_copy(dsb0[:, a:b], ot0[HD:128, a:b])
                        nc.vector.tensor_copy(dsb1[:, a:b], ot1[HD:128, a:b])
                        nc.vector.reciprocal_approx_fast(
                            out=rsb0[:, a:b], in_=dsb0[:, a:b])
                        nc.vector.reciprocal_approx_fast(
                            out=rsb1[:, a:b], in_=dsb1[:, a:b])
                        nc.vector.tensor_tensor(
                            out=ao[j][0:HD, a:b], in0=ot0[0:HD, a:b],
                            in1=rsb0[:, a:b], op=ALU.mult)
                        nc.vector.tensor_tensor(
                            out=ao[j][HD:128, a:b], in0=ot1[0:HD, a:b],
                            in1=rsb1[:, a:b], op=ALU.mult)
                        if final3:
                            emit_final_outproj_half(
                                ao, 3, qts=(0, 1) if si == 0 else (2, 3))
                    while emitted < len(filler):
                        filler[emitted]()
                        emitted += 1
                    if c == NQC - 1 and j == 2:
                        emit_final_outproj_half(ao, 2)
            # final chunk's out-projection is emitted inside the j==2/j==3
            # iterations via emit_final_outproj_half

    nc.compile()
    return nc


def _get_nc():
    if 'nc' not in _nc_cache:
        _nc_cache['nc'] = _build_nc()
    return _nc_cache['nc']


def kernel(x, w_qkv, w_out, _profile=False):
    from concourse.bass_utils import run_bass_kernel_spmd
    import ml_dtypes

    bf16 = ml_dtypes.bfloat16
    x = np.asarray(x, dtype=np.float32)
    w_qkv = np.asarray(w_qkv, dtype=np.float32)
    w_out = np.asarray(w_out, dtype=np.float32)

    nc = _get_nc()

    scale = np.float32(1.0 / np.sqrt(HD))
    in_maps = []
    for c in range(N_CORES):
        b, hh = c // 2, c % 2
        s, e = hh * DPC, (hh + 1) * DPC
        in_maps.append({
            'xT': np.ascontiguousarray(x[b].T).astype(bf16),
            'wq': np.ascontiguousarray(w_qkv[:, s:e] * scale).astype(bf16),
            'wk': np.ascontiguousarray(w_qkv[:, D+s:D+e]).astype(bf16),
            'wv': np.ascontiguousarray(w_qkv[:, 2*D+s:2*D+e]).astype(bf16),
            'wo': np.ascontiguousarray(w_out[s:e, :]).astype(bf16),
        })

    res = run_bass_kernel_spmd(nc, in_maps, core_ids=list(range(N_CORES)),
                               trace=_profile)
    out = np.empty((B, T, D), np.float32)
    for b in range(B):
        r0, r1 = res.results[2*b], res.results[2*b+1]
        out[b] = r0['po'].astype(np.float32) + r1['po'].astype(np.float32)
        out[b][T - 512:] += r0['pb'].astype(np.float32) + r1['pb'].astype(np.float32)
    if _profile:
        return out, res
    return out

